# revision 11
# baseline (speedup 1.0000x reference)
"""Bidirectional Mamba2 block, fused single-launch kernel on 8 NeuronCores.

Sharding: core_id = batch*4 + dir*2 + tokenhalf. Each core runs ALL 32 heads
for its 512 tokens: in_proj, causal depthwise conv (3-token halo is
host-precomputed), a chunked (SSD) selective scan, gated RMSNorm, out_proj,
and its direction's block of the final Wo matmul. The boundary SSM state
crosses the token-split via one masked pair AllReduce; prefix-state
corrections are applied late so the collective hides under compute.

Scheduling restructure vs v1: the dt projection + decay chain is emitted
FIRST so the DVE/ACT decay work overlaps the in_proj GEMMs; the SSD
W-matrix construction is pipelined DMA->DVE->Pool->ACT under the z-proj
GEMMs; the per-head D-skip matmuls are replaced by one broadcast multiply;
decay tensors live in a [c*32+h, q] packed layout so every elementwise op
runs on all 128 partitions; Loop3 (state corrections) and Loop4
(out_proj+Wo) are interleaved per chunk with per-chunk RMS factors.

Matmuls run in bf16 with fp32 PSUM accumulation; decay cumsums are fp32
(staged to DRAM as fp16/bf16 for the stride-0 partition-broadcast reads).
"""
import numpy as np
import ml_dtypes

import concourse.bass as bass
import concourse.bacc as bacc
import concourse.mybir as mybir
import concourse.tile as tile
from concourse import bass_utils

BF16 = ml_dtypes.bfloat16
F32 = mybir.dt.float32
F16 = mybir.dt.float16
BF = mybir.dt.bfloat16
AF = mybir.ActivationFunctionType
OP = mybir.AluOpType

L = 1024          # full sequence length
T = 512           # tokens per core
DM = 1024         # d_model
HN = 32           # heads per core (all of them)
NJ = HN // 2      # pair-packed head pairs
Q = 128           # chunk length
NCH = T // Q      # 4 chunks per core
CONVT = 17        # conv channel tiles: 16 xs + 1 (B|C)
NKT = DM // 128   # 8 contraction tiles over d_model
PAIR_GROUPS = [[0, 1], [2, 3], [4, 5], [6, 7]]


def build_core():
    nc = bacc.Bacc()

    xT_d = nc.dram_tensor("xT", [NKT, 128, T], BF, kind="ExternalInput")
    xbch_d = nc.dram_tensor("xbch", [CONVT, 128, 3], BF, kind="ExternalInput")
    wxbc_d = nc.dram_tensor("wxbc", [NKT, 128, CONVT * 128], BF, kind="ExternalInput")
    wz_d = nc.dram_tensor("wz", [NKT, 128, 2048], BF, kind="ExternalInput")
    wdt_d = nc.dram_tensor("wdt", [NKT, 128, HN], BF, kind="ExternalInput")
    wout_d = nc.dram_tensor("wout", [16, 128, 1024], BF, kind="ExternalInput")
    wo_d = nc.dram_tensor("wo", [8, 128, 1024], BF, kind="ExternalInput")
    cwT_d = nc.dram_tensor("cwT", [128, 4, CONVT], F32, kind="ExternalInput")
    convb_d = nc.dram_tensor("convb", [CONVT, 128], F32, kind="ExternalInput")
    dtb_d = nc.dram_tensor("dtb", [128], F32, kind="ExternalInput")
    negA_d = nc.dram_tensor("negA", [128], F32, kind="ExternalInput")
    Dexp_d = nc.dram_tensor("Dexp", [2048], BF, kind="ExternalInput")
    tri01_d = nc.dram_tensor("tri01", [128, 128], BF, kind="ExternalInput")
    idf_d = nc.dram_tensor("idf", [128, 128], F32, kind="ExternalInput")
    idb_d = nc.dram_tensor("idb", [128, 128], BF, kind="ExternalInput")
    smask_d = nc.dram_tensor("smask", [128, 1], F32, kind="ExternalInput")
    rmask_d = nc.dram_tensor("rmask", [128, 1], F32, kind="ExternalInput")

    # decay tensors in DRAM for stride-0 partition-broadcast reads,
    # packed [c*32+h, q]
    ecum_dram = nc.dram_tensor("ecum_dram", [128, Q], BF)
    cumh_dram = nc.dram_tensor("cumh_dram", [128, Q], F16)
    C_d = nc.dram_tensor("C", [NCH, 128, 1024], F32, kind="ExternalOutput")

    with tile.TileContext(nc) as tc:
        with (
            tc.tile_pool(name="res", bufs=1) as res,
            tc.tile_pool(name="work", bufs=2) as work,
            tc.tile_pool(name="wx", bufs=2) as wx,
            tc.tile_pool(name="dram", bufs=1, space="DRAM") as dram,
            tc.tile_pool(name="ps_main", bufs=3, space="PSUM") as ps_main,
            tc.tile_pool(name="ps_tr", bufs=2, space="PSUM") as ps_tr,
            tc.tile_pool(name="ps_y", bufs=2, space="PSUM") as ps_y,
        ):
            # ---- resident inputs (order matters: dt path first) ------------
            idb = res.tile([128, 128], BF)
            nc.sync.dma_start(idb[:], idb_d[:, :])
            xT = res.tile([128, NKT, T], BF)
            nc.sync.dma_start(xT[:], xT_d.rearrange("k p t -> p k t"))
            wdt = res.tile([128, NKT, HN], BF)
            nc.sync.dma_start(wdt[:], wdt_d.rearrange("k p t -> p k t"))
            dtb4 = res.tile([128, 1], F32)
            nc.sync.dma_start(dtb4[:], dtb_d.rearrange("(h o) -> h o", o=1))
            negA4 = res.tile([128, 1], F32)
            nc.sync.dma_start(negA4[:], negA_d.rearrange("(h o) -> h o", o=1))
            idf = res.tile([128, 128], F32)
            nc.scalar.dma_start(idf[:], idf_d[:, :])
            cwT = res.tile([128, 4, CONVT], F32)
            nc.scalar.dma_start(cwT[:], cwT_d[:, :, :])
            convb = res.tile([128, CONVT], F32)
            nc.scalar.dma_start(convb[:], convb_d.rearrange("k p -> p k"))
            tri01 = res.tile([128, 128], BF)
            nc.scalar.dma_start(tri01[:], tri01_d[:, :])
            smask = res.tile([128, 1], F32)
            nc.scalar.dma_start(smask[:], smask_d[:, :])
            rmask = res.tile([128, 1], F32)
            nc.scalar.dma_start(rmask[:], rmask_d[:, :])
            Dexp = res.tile([128, 2048], BF)
            nc.sync.dma_start(
                Dexp[:], bass.AP(tensor=Dexp_d, offset=0, ap=[[0, 128], [1, 2048]]))
            wxbc = res.tile([128, NKT, CONVT * 128], BF, tag="w1")
            wxbc_r = wxbc_d.rearrange("k p t -> p k t")
            for ct in range(CONVT):
                eng = nc.sync if ct % 2 == 0 else nc.scalar
                eng.dma_start(wxbc[:, :, ct * 128:(ct + 1) * 128],
                              wxbc_r[:, :, ct * 128:(ct + 1) * 128])
            wz = res.tile([128, NKT, 2048], BF, tag="w2")
            wz_r = wz_d.rearrange("k p t -> p k t")
            for nt in range(4):
                eng = nc.sync if nt % 2 == 0 else nc.scalar
                eng.dma_start(wz[:, :, nt * 512:(nt + 1) * 512],
                              wz_r[:, :, nt * 512:(nt + 1) * 512])

            # ---- resident intermediates ------------------------------------
            xbcT = res.tile([128, CONVT, 3 + T], BF, tag="bigT")
            nc.sync.dma_start(xbcT[:, :, 0:3], xbch_d.rearrange("c p t -> p c t"))
            BCt = res.tile([128, T], BF)        # 0:64 B^T, 64:128 C^T
            Ct2 = res.tile([128, T], BF)        # C^T duplicated on both halves
            xsTok = res.tile([128, NCH, 2048], BF)
            gb = res.tile([128, NCH, 2048], BF)          # z, then gated y
            Btok = res.tile([128, NCH, 64], BF)
            # decay tensors, packed layout [c*32+h, q]
            dt4 = res.tile([128, Q], F32)       # raw dt proj, then softplus
            lndt = res.tile([128, Q], F32)
            cum = res.tile([128, Q], F32)
            clg = res.tile([128, Q], F32)       # cum - ln dt
            negclT = res.tile([128, 128], F16)  # [q, c*32+h] = ln dt - cum
            dstateT = res.tile([128, 128], BF)   # [q, c*32+h]
            lam = res.tile([128, NJ, NCH], BF)   # pair-packed chunk decay
            Pp = res.tile([128, NCH, NJ], F32)   # cumulative lam products
            Snp = res.tile([128, NCH * NJ, 64], BF)
            yi = res.tile([128, NCH, 2048], BF, tag="bigT")  # intra-chunk y
            ss_sb = res.tile([128, NCH], F32)
            rs_sb = res.tile([128, NCH], F32)
            ones4 = res.tile([128, Q], F32)
            nc.vector.memset(ones4[:], 1.0)

            # ---- dt projection + decay chain (emitted FIRST) ---------------
            pdt = ps_main.tile([HN, T], F32, tag="mm", name="pdt")
            for kt in range(NKT):
                nc.tensor.matmul(pdt[:], wdt[:, kt, :], xT[:, kt, :],
                                 start=(kt == 0), stop=(kt == NKT - 1))
            dtsb = work.tile([HN, T], F32, tag="dtsb", bufs=1, name="dtsb")
            nc.scalar.copy(dtsb[:], pdt[:])
            for c in range(NCH):
                nc.sync.dma_start(dt4[c * 32:(c + 1) * 32, :],
                                  dtsb[:, c * Q:(c + 1) * Q])
            # softplus: dt = ln(1 + exp(raw + dtb))
            nc.scalar.activation(dt4[:], dt4[:], AF.Exp, bias=dtb4[:])
            nc.scalar.activation(dt4[:], dt4[:], AF.Ln, bias=1.0)
            nc.scalar.activation(lndt[:], dt4[:], AF.Ln)
            nc.vector.tensor_scalar_mul(dt4[:], dt4[:], negA4[:])
            nc.vector.tensor_tensor_scan(
                cum[:], ones4[:], dt4[:], 0.0, OP.mult, OP.add)
            nc.vector.tensor_tensor(clg[:], cum[:], lndt[:], OP.subtract)
            ptr_cl = ps_tr.tile([128, 128], F32, tag="trd", bufs=1, name="ptr_cl")
            nc.tensor.transpose(ptr_cl[:], clg[:], idf[:])
            nc.vector.tensor_scalar_mul(negclT[:], ptr_cl[:], -1.0)
            dscm = work.tile([128, Q], F32, tag="dscm", bufs=1, name="dscm")
            nc.scalar.activation(dscm[:], clg[:], AF.Exp,
                                 bias=cum[:, Q - 1:Q], scale=-1.0)
            ptr_ds = ps_tr.tile([128, 128], F32, tag="trd", bufs=1, name="ptr_ds")
            nc.tensor.transpose(ptr_ds[:], dscm[:], idf[:])
            nc.vector.tensor_copy(dstateT[:], ptr_ds[:])
            ecum_sb = work.tile([128, Q], BF, tag="ecum", bufs=1, name="ecum_sb")
            nc.scalar.activation(ecum_sb[:], cum[:], AF.Exp)
            nc.sync.dma_start(ecum_dram[:, :], ecum_sb[:])
            cum16 = work.tile([128, Q], F16, tag="cum16", bufs=1, name="cum16")
            nc.vector.tensor_copy(cum16[:], cum[:])
            nc.sync.dma_start(cumh_dram[:, :], cum16[:])

            # lam[p, j, c] = ecum[c*32 + j (+16 hi-half), Q-1] via stride-0 DMA
            for c in range(NCH):
                nc.sync.dma_start(
                    lam[0:64, :, c],
                    bass.AP(tensor=ecum_dram, offset=c * 32 * Q + Q - 1,
                            ap=[[0, 64], [Q, NJ]]))
                nc.sync.dma_start(
                    lam[64:128, :, c],
                    bass.AP(tensor=ecum_dram, offset=(c * 32 + 16) * Q + Q - 1,
                            ap=[[0, 64], [Q, NJ]]))
            nc.vector.tensor_copy(Pp[:, 0, :], lam[:, :, 0])
            for c in range(1, NCH):
                nc.vector.tensor_tensor(Pp[:, c, :], Pp[:, c - 1, :],
                                        lam[:, :, c], OP.mult)

            # ---- in_proj: xBC block (channel-major) ------------------------
            for ct in range(CONVT):
                pt = ps_main.tile([128, T], F32, tag="mm", name="pxbc")
                for kt in range(NKT):
                    nc.tensor.matmul(
                        pt[:], wxbc[:, kt, ct * 128:(ct + 1) * 128],
                        xT[:, kt, :], start=(kt == 0), stop=(kt == NKT - 1))
                nc.vector.tensor_copy(xbcT[:, ct, 3:3 + T], pt[:])

            # ---- conv via PE diagonal matmuls + silu -----------------------
            # diagonal conv-weight tiles built on device from the identity
            conv_out = []
            for ct in range(CONVT):
                dw = wx.tile([128, 4, 128], BF, tag="dw", bufs=2, name="dw")
                for k in range(4):
                    nc.vector.tensor_scalar_mul(dw[:, k, :], idb[:],
                                                cwT[:, k, ct:ct + 1])
                cp = ps_main.tile([128, T], F32, tag="mm", name="pconv")
                for k in range(4):
                    nc.tensor.matmul(cp[:], dw[:, k, :],
                                     xbcT[:, ct, k:k + T],
                                     start=(k == 0), stop=(k == 3))
                dest = BCt
                if ct < 16:
                    xs_scr = wx.tile([128, T], BF, tag="xs_scr")
                    conv_out.append(xs_scr)
                    dest = xs_scr
                nc.scalar.activation(dest[:], cp[:], AF.Silu,
                                     bias=convb[:, ct:ct + 1])
            nc.sync.dma_start(Ct2[0:64, :], BCt[64:128, :])
            nc.sync.dma_start(Ct2[64:128, :], BCt[64:128, :])

            # wout aliases wxbc (last read by in_proj); load as soon as freed
            wout = res.tile([128, 16, 1024], BF, tag="w1")
            wout_r = wout_d.rearrange("k p t -> p k t")
            for i in range(4):
                eng = nc.sync if i % 2 == 0 else nc.scalar
                eng.dma_start(wout[:, i * 4:(i + 1) * 4, :],
                              wout_r[:, i * 4:(i + 1) * 4, :])

            # ---- xs transposes -> token-major xsTok ------------------------
            for ct in range(16):
                xs_scr = conv_out[ct]
                ptr = ps_tr.tile([128, 512], BF, tag="trb")
                for i in range(NCH):
                    nc.tensor.transpose(
                        ptr[:, i * 128:(i + 1) * 128],
                        xs_scr[:, i * 128:(i + 1) * 128], idb[:])
                nc.scalar.copy(
                    xsTok[:, :, ct * 128:(ct + 1) * 128],
                    ptr[:].rearrange("p (i q) -> p i q", i=NCH))
            # B transposes: [64, T] -> Btok [128, NCH, 64]
            ptrB = ps_tr.tile([128, 512], BF, tag="trb")
            for c in range(NCH):
                nc.tensor.transpose(
                    ptrB[:, c * 128:c * 128 + 64],
                    BCt[0:64, c * 128:(c + 1) * 128], idb[0:64, 0:64])
            nc.scalar.copy(
                Btok[:, :, :],
                ptrB[:].rearrange("p (i q) -> p i q", i=NCH)[:, :, 0:64])

            # ---- Loop1: local chunk states (zero entering state) -----------
            for c in range(NCH):
                Btil = work.tile([128, HN, 64], BF, tag="Btil", bufs=1, name="Btil")
                nc.vector.tensor_tensor(
                    Btil[:],
                    bass.AP(tensor=Btok.tensor, offset=Btok[:, c, :].offset,
                            ap=[Btok.ap[0], [0, HN], [1, 64]]),
                    bass.AP(tensor=dstateT.tensor,
                            offset=dstateT[:, c * 32:(c + 1) * 32].offset,
                            ap=[dstateT.ap[0], [1, HN], [0, 64]]),
                    OP.mult)
                for half in range(2):
                    pu = ps_y.tile([128, 8, 64], F32, tag="py", name="pu")
                    for j in range(8):
                        jj = half * 8 + j
                        for par in range(2):
                            h = 16 * par + jj
                            nc.tensor.matmul(
                                pu[par * 64:par * 64 + 64, j, :],
                                Btil[:, h, :],
                                xsTok[:, c, h * 64:(h + 1) * 64],
                                start=True, stop=True)
                    jsl = slice(half * 8, (half + 1) * 8)
                    if c == 0:
                        nc.vector.tensor_copy(Snp[:, jsl, :], pu[:])
                    else:
                        tmp = work.tile([128, 8, 64], BF, tag="stmp")
                        nc.vector.tensor_tensor(
                            tmp[:], Snp[:, (c - 1) * NJ + half * 8:(c - 1) * NJ + half * 8 + 8, :],
                            bass.AP(tensor=lam.tensor,
                                    offset=lam[:, jsl, c].offset,
                                    ap=[lam.ap[0], [NCH, 8], [0, 64]]),
                            OP.mult)
                        nc.vector.tensor_tensor(
                            Snp[:, c * NJ + half * 8:c * NJ + half * 8 + 8, :],
                            tmp[:], pu[:], OP.add)

            # ---- boundary state AllReduce over token-half pairs ------------
            with tc.high_priority():
                ar_in = work.tile([128, NJ * 64], BF, tag="ario", bufs=1,
                                  name="ar_in")
                nc.vector.tensor_scalar_mul(
                    ar_in[:], Snp[:, (NCH - 1) * NJ:NCH * NJ, :].rearrange("p j q -> p (j q)"),
                    smask[:])
                bb_in = dram.tile([128, NJ * 64], BF)
                bb_out = dram.tile([128, NJ * 64], BF)
                nc.gpsimd.dma_start(bb_in[:], ar_in[:])
                nc.gpsimd.collective_compute(
                    "AllReduce", OP.add, replica_groups=PAIR_GROUPS,
                    ins=[bb_in.opt()], outs=[bb_out.opt()])

            # ---- z proj + Loop2 (W construction + intra-chunk y),
            #      interleaved per chunk; hides the collective ---------------
            for c in range(NCH):
                for nt in range(4):
                    pz = ps_main.tile([128, T], F32, tag="mm", name="pz")
                    for kt in range(NKT):
                        nc.tensor.matmul(
                            pz[:], xT[:, kt, c * 128:(c + 1) * 128],
                            wz[:, kt, nt * 512:(nt + 1) * 512],
                            start=(kt == 0), stop=(kt == NKT - 1))
                    nc.vector.tensor_copy(gb[:, c, nt * 512:(nt + 1) * 512],
                                          pz[:])
                sl = slice(c * Q, (c + 1) * Q)
                pg = ps_y.tile([128, 128], F32, tag="py", name="pg")
                nc.tensor.matmul(pg[:], BCt[0:64, sl], Ct2[0:64, sl],
                                 start=True, stop=True)
                gsb = work.tile([128, 128], BF, tag="gsb", name="gsb")
                nc.vector.tensor_tensor(gsb[:], pg[:], tri01[:], OP.mult)
                Dxs = work.tile([128, 2048], BF, tag="dxs", bufs=1, name="Dxs")
                nc.vector.tensor_tensor(Dxs[:], xsTok[:, c, :], Dexp[:],
                                        OP.mult)
                crow = work.tile([128, HN, Q], F16, tag="crow", bufs=1,
                                 name="crow")
                nc.sync.dma_start(
                    crow[:],
                    bass.AP(tensor=cumh_dram, offset=c * 32 * Q,
                            ap=[[0, 128], [Q, HN], [1, Q]]))
                nc.vector.tensor_tensor(
                    crow[:], crow[:],
                    bass.AP(tensor=negclT.tensor,
                            offset=negclT[:, c * 32:(c + 1) * 32].offset,
                            ap=[negclT.ap[0], [1, HN], [0, Q]]),
                    OP.add)
                WT = work.tile([128, HN, Q], BF, tag="wt", bufs=1, name="WT")
                nc.vector.tensor_scalar_min(WT[:], crow[:], 30.0)
                nc.scalar.activation(WT[:], WT[:], AF.Exp)
                nc.vector.tensor_tensor(
                    WT[:],
                    bass.AP(tensor=gsb.tensor, offset=gsb.offset,
                            ap=[gsb.ap[0], [0, HN], gsb.ap[1]]),
                    WT[:], OP.mult)
                for hh in range(4):
                    py = ps_y.tile([128, 8, 64], F32, tag="py", name="py")
                    for i in range(8):
                        h = hh * 8 + i
                        nc.tensor.matmul(py[:, i, :], WT[:, h, :],
                                         xsTok[:, c, h * 64:(h + 1) * 64],
                                         start=True, stop=True)
                    nc.vector.tensor_tensor(
                        yi[:, c, hh * 512:(hh + 1) * 512],
                        py[:].rearrange("p j q -> p (j q)"),
                        Dxs[:, hh * 512:(hh + 1) * 512], OP.add)

            # wo aliases wz (last read by the z proj); load as soon as freed
            wo = res.tile([128, 8, 1024], BF, tag="w2")
            wo_r = wo_d.rearrange("k p t -> p k t")
            for i in range(2):
                nc.sync.dma_start(wo[:, i * 4:(i + 1) * 4, :],
                                  wo_r[:, i * 4:(i + 1) * 4, :])

            # ---- collective result: entering state per chunk ---------------
            with tc.high_priority():
                s_in = work.tile([128, NJ * 64], BF, tag="sins", bufs=1,
                                 name="s_in")
                nc.sync.dma_start(s_in[:], bb_out[:])
                seff = work.tile([128, NJ, 64], BF, tag="seff", bufs=1,
                                 name="seff")
                nc.vector.tensor_scalar_mul(
                    seff[:], s_in[:].rearrange("p (j q) -> p j q", j=NJ),
                    rmask[:])
                # Snp[c] <- Snp[c-1] + Pp[c-1] * seff ; Snp[0] <- seff
                for c in range(NCH - 1, 0, -1):
                    tmp = work.tile([128, NJ, 64], BF, tag="sutmp", bufs=1)
                    nc.vector.tensor_tensor(
                        tmp[:], seff[:],
                        bass.AP(tensor=Pp.tensor, offset=Pp[:, c - 1, :].offset,
                                ap=[Pp.ap[0], [1, NJ], [0, 64]]),
                        OP.mult)
                    nc.vector.tensor_tensor(Snp[:, c * NJ:(c + 1) * NJ, :],
                                            tmp[:],
                                            Snp[:, (c - 1) * NJ:c * NJ, :],
                                            OP.add)
                nc.vector.tensor_copy(Snp[:, 0:NJ, :], seff[:])

            # ---- Loop3+Loop4 per chunk: corrections, gating, out_proj, Wo --
            for c in range(NCH):
                sl = slice(c * Q, (c + 1) * Q)
                # Ctilde pair-packed: C^T * exp(cum) per head
                ecrow = work.tile([128, NJ, Q], BF, tag="ecrow", bufs=1, name="ecrow")
                nc.sync.dma_start(
                    ecrow[0:64, :, :],
                    bass.AP(tensor=ecum_dram, offset=c * 32 * Q,
                            ap=[[0, 64], [Q, NJ], [1, Q]]))
                nc.sync.dma_start(
                    ecrow[64:128, :, :],
                    bass.AP(tensor=ecum_dram, offset=(c * 32 + 16) * Q,
                            ap=[[0, 64], [Q, NJ], [1, Q]]))
                Ctil = ecrow
                nc.vector.tensor_tensor(
                    Ctil[:],
                    bass.AP(tensor=Ct2.tensor, offset=Ct2[:, sl].offset,
                            ap=[Ct2.ap[0], [0, NJ], [1, Q]]),
                    ecrow[:], OP.mult)
                sgz = work.tile([128, 2048], BF, tag="sgz", bufs=1, name="sgz")
                nc.scalar.activation(sgz[:], gb[:, c, :], AF.Silu)
                for hh in range(4):
                    py2 = ps_y.tile([128, 8, 64], F32, tag="py", name="py2")
                    for i in range(8):
                        h = hh * 8 + i
                        par = h // 16
                        nc.tensor.matmul(
                            py2[:, i, :],
                            Ctil[par * 64:par * 64 + 64, h % 16, :],
                            Snp[par * 64:par * 64 + 64, c * NJ + h % 16, :],
                            start=True, stop=True)
                    hsl = slice(hh * 512, (hh + 1) * 512)
                    ysum = work.tile([128, 512], BF, tag="ysum", name="ysum")
                    nc.vector.tensor_tensor(
                        ysum[:], py2[:].rearrange("p j q -> p (j q)"),
                        yi[:, c, hsl], OP.add)
                    nc.vector.tensor_tensor(gb[:, c, hsl], ysum[:],
                                            sgz[:, hsl], OP.mult)
                sq = work.tile([128, 2048], BF, tag="dxs", bufs=1, name="sq")
                nc.vector.scalar_tensor_tensor(
                    sq[:], gb[:, c, :], 1.0, gb[:, c, :], OP.mult, OP.mult,
                    accum_out=ss_sb[:, c:c + 1])
                # per-chunk rs = 1/sqrt(mean(y^2) + eps)
                nc.vector.tensor_scalar(rs_sb[:, c:c + 1], ss_sb[:, c:c + 1],
                                        1.0 / 2048.0, 1e-5, OP.mult, OP.add)
                nc.scalar.activation(rs_sb[:, c:c + 1], rs_sb[:, c:c + 1],
                                     AF.Sqrt)
                nc.vector.reciprocal(rs_sb[:, c:c + 1], rs_sb[:, c:c + 1])

                # out_proj: transpose gb chunk, matmul with wout, scale by rs
                gbt = work.tile([128, 16, 128], BF, tag="gbt", bufs=1, name="gbt")
                for pg_i in range(4):
                    ptr = ps_tr.tile([128, 512], BF, tag="trb")
                    for i in range(4):
                        hp = pg_i * 4 + i
                        nc.tensor.transpose(
                            ptr[:, i * 128:(i + 1) * 128],
                            gb[:, c, hp * 128:(hp + 1) * 128], idb[:])
                    nc.vector.tensor_copy(
                        gbt[:, pg_i * 4:(pg_i + 1) * 4, :],
                        ptr[:].rearrange("p (i q) -> p i q", i=4))
                Pn = work.tile([128, 1024], BF, tag="Pn", bufs=1, name="Pn")
                for nt in range(2):
                    po = ps_main.tile([128, T], F32, tag="mm", name="po")
                    for kt in range(16):
                        nc.tensor.matmul(
                            po[:], gbt[:, kt, :],
                            wout[:, kt, nt * 512:(nt + 1) * 512],
                            start=(kt == 0), stop=(kt == 15))
                    nc.vector.tensor_scalar_mul(
                        Pn[:, nt * 512:(nt + 1) * 512], po[:],
                        rs_sb[:, c:c + 1])
                # Wo block: transpose Pn, matmul with wo
                pnT = work.tile([128, 8, 128], BF, tag="gbt", bufs=1, name="pnT")
                for pg_i in range(2):
                    ptr = ps_tr.tile([128, 512], BF, tag="trb")
                    for i in range(4):
                        hp = pg_i * 4 + i
                        nc.tensor.transpose(
                            ptr[:, i * 128:(i + 1) * 128],
                            Pn[:, hp * 128:(hp + 1) * 128], idb[:])
                    nc.scalar.copy(
                        pnT[:, pg_i * 4:(pg_i + 1) * 4, :],
                        ptr[:].rearrange("p (i q) -> p i q", i=4))
                for nt in range(2):
                    co = ps_main.tile([128, T], F32, tag="mm", name="co")
                    for kt in range(8):
                        nc.tensor.matmul(
                            co[:], pnT[:, kt, :],
                            wo[:, kt, nt * 512:(nt + 1) * 512],
                            start=(kt == 0), stop=(kt == 7))
                    cof = work.tile([128, 512], F32, tag="cof", bufs=1, name="cof")
                    nc.vector.tensor_copy(cof[:], co[:])
                    nc.sync.dma_start(C_d[c, :, nt * 512:(nt + 1) * 512],
                                      cof[:])

    nc.compile()
    return nc


# ----------------------------------------------------------------------------
# Host orchestration
# ----------------------------------------------------------------------------
_cache = {}


def _get_core_nc():
    if "core" not in _cache:
        _cache["core"] = build_core()
    return _cache["core"]


def _prep_core_inputs(x_seq, Win, convw, convb, dtb, Alog, Dsk, normw, Wout,
                      Wo, d_idx, h):
    """Host-side shard/cast for one (dir, batch, token-half) core."""
    z_cols = np.arange(0, 2048)
    xs_cols = np.arange(2048, 4096)
    bc_cols = np.arange(4096, 4224)
    dt_cols = np.arange(4224, 4256)
    xsbc_cols = np.concatenate([xs_cols, bc_cols])

    toks = x_seq[h * T:(h + 1) * T]                             # [512, 1024]
    if h == 0:
        xbch = np.zeros((3, CONVT * 128), np.float32)
    else:
        xbch = x_seq[h * T - 3:h * T] @ Win[:, xsbc_cols]       # [3, 2176]

    wxbc = Win[:, xsbc_cols]                                    # [1024, 2176]
    wz = Win[:, z_cols]
    wdt = Win[:, dt_cols]
    wout = normw[:, None] * Wout                                # [2048, 1024]
    wo_blk = Wo[d_idx * 1024:(d_idx + 1) * 1024]                # [1024, 1024]

    cw = convw[:, 0, :]                                         # [4, 2176]
    cb = convb                                                  # [2176]
    cwT = np.ascontiguousarray(
        cw.reshape(4, CONVT, 128).transpose(2, 0, 1))           # [128, 4, 17]
    tri01 = (np.arange(128)[None, :] >= np.arange(128)[:, None]).astype(BF16)
    mval = 1.0 if h == 0 else 0.0

    return {
        "xT": np.ascontiguousarray(toks.T.reshape(NKT, 128, T).astype(BF16)),
        "xbch": np.ascontiguousarray(
            xbch.T.reshape(CONVT, 128, 3).astype(BF16)),
        "wxbc": np.ascontiguousarray(
            wxbc.reshape(NKT, 128, CONVT * 128).astype(BF16)),
        "wz": np.ascontiguousarray(wz.reshape(NKT, 128, 2048).astype(BF16)),
        "wdt": np.ascontiguousarray(wdt.reshape(NKT, 128, HN).astype(BF16)),
        "wout": np.ascontiguousarray(wout.reshape(16, 128, 1024).astype(BF16)),
        "wo": np.ascontiguousarray(wo_blk.reshape(8, 128, 1024).astype(BF16)),
        "cwT": cwT.astype(np.float32),
        "convb": np.ascontiguousarray(cb.reshape(CONVT, 128)).astype(np.float32),
        "dtb": np.tile(np.asarray(dtb, np.float32), NCH),
        "negA": np.tile((-np.exp(Alog)).astype(np.float32), NCH),
        "Dexp": np.repeat(np.asarray(Dsk, np.float32), 64).astype(BF16),
        "tri01": tri01,
        "idf": np.eye(128, dtype=np.float32),
        "idb": np.eye(128, dtype=np.float32).astype(BF16),
        "smask": np.full((128, 1), mval, np.float32),
        "rmask": np.full((128, 1), 1.0 - mval, np.float32),
    }


def _build_in_maps(inputs):
    x = np.asarray(inputs["x"], dtype=np.float32)               # [2, L, DM]
    Wo = np.asarray(inputs["Wo"], dtype=np.float32)
    dir_params = {}
    for pre in ("f", "b"):
        dir_params[pre] = tuple(np.asarray(inputs[pre + k], dtype=np.float32)
                                for k in ("Win", "convw", "convb", "dtb",
                                          "Alog", "D", "normw", "Wout"))
    # core order: id = b*4 + d*2 + h
    core_keys = [(b, d, h) for b in range(2) for d in range(2)
                 for h in range(2)]
    in_maps = []
    for (b, d, h) in core_keys:
        pre = "f" if d == 0 else "b"
        x_seq = x[b] if d == 0 else x[b, ::-1]
        in_maps.append(_prep_core_inputs(x_seq, *dir_params[pre], Wo, d, h))
    return in_maps, core_keys


def kernel(**inputs):
    in_maps, core_keys = _build_in_maps(inputs)
    nc = _get_core_nc()
    res = bass_utils.run_bass_kernel_spmd(nc, in_maps, core_ids=list(range(8)))

    bo = np.asarray(inputs["bo"], dtype=np.float32)
    out = np.zeros((2, L, DM), np.float32)
    for ci, (b, d, h) in enumerate(core_keys):
        Cp = res.results[ci]["C"].reshape(T, 1024)
        if d == 0:
            out[b, h * T:(h + 1) * T] += Cp
        else:
            # bwd core's tokens are reversed-sequence [h*T:(h+1)*T]
            out[b, L - (h + 1) * T:L - h * T] += Cp[::-1]
    out += bo[None, None, :]
    return out.astype(np.float32)


# revision 12
# speedup vs baseline: 1.0651x; 1.0651x over previous
"""Bidirectional Mamba2 block, fused single-launch kernel on 8 NeuronCores.

Sharding: core_id = batch*4 + dir*2 + tokenhalf. Each core runs ALL 32 heads
for its 512 tokens: in_proj, causal depthwise conv (3-token halo is
host-precomputed), a chunked (SSD) selective scan, gated RMSNorm, out_proj,
and its direction's block of the final Wo matmul. The boundary SSM state
crosses the token-split via one masked pair AllReduce; prefix-state
corrections are applied late so the collective hides under compute.

Scheduling restructure vs v1: the dt projection + decay chain is emitted
FIRST so the DVE/ACT decay work overlaps the in_proj GEMMs; the SSD
W-matrix construction is pipelined DMA->DVE->Pool->ACT under the z-proj
GEMMs; the per-head D-skip matmuls are replaced by one broadcast multiply;
decay tensors live in a [c*32+h, q] packed layout so every elementwise op
runs on all 128 partitions; Loop3 (state corrections) and Loop4
(out_proj+Wo) are interleaved per chunk with per-chunk RMS factors.

Matmuls run in bf16 with fp32 PSUM accumulation; decay cumsums are fp32
(staged to DRAM as fp16/bf16 for the stride-0 partition-broadcast reads).
"""
import numpy as np
import ml_dtypes

import concourse.bass as bass
import concourse.bacc as bacc
import concourse.mybir as mybir
import concourse.tile as tile
from concourse import bass_utils

BF16 = ml_dtypes.bfloat16
F32 = mybir.dt.float32
F16 = mybir.dt.float16
BF = mybir.dt.bfloat16
AF = mybir.ActivationFunctionType
OP = mybir.AluOpType

L = 1024          # full sequence length
T = 512           # tokens per core
DM = 1024         # d_model
HN = 32           # heads per core (all of them)
NJ = HN // 2      # pair-packed head pairs
Q = 128           # chunk length
NCH = T // Q      # 4 chunks per core
CONVT = 17        # conv channel tiles: 16 xs + 1 (B|C)
NKT = DM // 128   # 8 contraction tiles over d_model
PAIR_GROUPS = [[0, 1], [2, 3], [4, 5], [6, 7]]


def build_core():
    nc = bacc.Bacc()

    xT_d = nc.dram_tensor("xT", [NKT, 128, T], BF, kind="ExternalInput")
    xbch_d = nc.dram_tensor("xbch", [CONVT, 128, 3], BF, kind="ExternalInput")
    wxbc_d = nc.dram_tensor("wxbc", [NKT, 128, CONVT * 128], BF, kind="ExternalInput")
    wz_d = nc.dram_tensor("wz", [NKT, 128, 2048], BF, kind="ExternalInput")
    wdt_d = nc.dram_tensor("wdt", [NKT, 128, HN], BF, kind="ExternalInput")
    wout_d = nc.dram_tensor("wout", [16, 128, 1024], BF, kind="ExternalInput")
    wo_d = nc.dram_tensor("wo", [8, 128, 1024], BF, kind="ExternalInput")
    cwT_d = nc.dram_tensor("cwT", [128, 4, CONVT], F32, kind="ExternalInput")
    convb_d = nc.dram_tensor("convb", [CONVT, 128], F32, kind="ExternalInput")
    dtb_d = nc.dram_tensor("dtb", [128], F32, kind="ExternalInput")
    negA_d = nc.dram_tensor("negA", [128], F32, kind="ExternalInput")
    Dexp_d = nc.dram_tensor("Dexp", [2048], BF, kind="ExternalInput")
    tri01_d = nc.dram_tensor("tri01", [128, 128], BF, kind="ExternalInput")
    idf_d = nc.dram_tensor("idf", [128, 128], F32, kind="ExternalInput")
    idb_d = nc.dram_tensor("idb", [128, 128], BF, kind="ExternalInput")
    smask_d = nc.dram_tensor("smask", [128, 1], F32, kind="ExternalInput")
    rmask_d = nc.dram_tensor("rmask", [128, 1], F32, kind="ExternalInput")

    # decay tensors in DRAM for stride-0 partition-broadcast reads,
    # packed [c*32+h, q]
    ecum_dram = nc.dram_tensor("ecum_dram", [128, Q], BF)
    cumh_dram = nc.dram_tensor("cumh_dram", [128, Q], F16)
    C_d = nc.dram_tensor("C", [NCH, 128, 1024], F32, kind="ExternalOutput")

    with tile.TileContext(nc) as tc:
        with (
            tc.tile_pool(name="res", bufs=1) as res,
            tc.tile_pool(name="work", bufs=2) as work,
            tc.tile_pool(name="wx", bufs=2) as wx,
            tc.tile_pool(name="dram", bufs=1, space="DRAM") as dram,
            tc.tile_pool(name="ps_main", bufs=3, space="PSUM") as ps_main,
            tc.tile_pool(name="ps_tr", bufs=2, space="PSUM") as ps_tr,
            tc.tile_pool(name="ps_y", bufs=2, space="PSUM") as ps_y,
        ):
            # ---- resident inputs (order matters: dt path first) ------------
            idb = res.tile([128, 128], BF)
            nc.sync.dma_start(idb[:], idb_d[:, :])
            xT = res.tile([128, NKT, T], BF)
            nc.sync.dma_start(xT[:], xT_d.rearrange("k p t -> p k t"))
            wdt = res.tile([128, NKT, HN], BF)
            nc.sync.dma_start(wdt[:], wdt_d.rearrange("k p t -> p k t"))
            dtb4 = res.tile([128, 1], F32)
            nc.sync.dma_start(dtb4[:], dtb_d.rearrange("(h o) -> h o", o=1))
            negA4 = res.tile([128, 1], F32)
            nc.sync.dma_start(negA4[:], negA_d.rearrange("(h o) -> h o", o=1))
            idf = res.tile([128, 128], F32)
            nc.scalar.dma_start(idf[:], idf_d[:, :])
            cwT = res.tile([128, 4, CONVT], F32)
            nc.scalar.dma_start(cwT[:], cwT_d[:, :, :])
            convb = res.tile([128, CONVT], F32)
            nc.scalar.dma_start(convb[:], convb_d.rearrange("k p -> p k"))
            tri01 = res.tile([128, 128], BF)
            nc.scalar.dma_start(tri01[:], tri01_d[:, :])
            smask = res.tile([128, 1], F32)
            nc.scalar.dma_start(smask[:], smask_d[:, :])
            rmask = res.tile([128, 1], F32)
            nc.scalar.dma_start(rmask[:], rmask_d[:, :])
            Dexp = res.tile([128, 2048], BF)
            nc.sync.dma_start(
                Dexp[:], bass.AP(tensor=Dexp_d, offset=0, ap=[[0, 128], [1, 2048]]))
            wxbc = res.tile([128, NKT, CONVT * 128], BF, tag="w1")
            wxbc_r = wxbc_d.rearrange("k p t -> p k t")
            for ct in range(CONVT):
                nc.sync.dma_start(wxbc[:, :, ct * 128:(ct + 1) * 128],
                                  wxbc_r[:, :, ct * 128:(ct + 1) * 128])
            wz = res.tile([128, NKT, 2048], BF, tag="w2")
            wz_r = wz_d.rearrange("k p t -> p k t")
            for nt in range(4):
                nc.sync.dma_start(wz[:, :, nt * 512:(nt + 1) * 512],
                                  wz_r[:, :, nt * 512:(nt + 1) * 512])

            # ---- resident intermediates ------------------------------------
            xbcT = res.tile([128, CONVT, 3 + T], BF, tag="bigT")
            nc.sync.dma_start(xbcT[:, :, 0:3], xbch_d.rearrange("c p t -> p c t"))
            BCt = res.tile([128, T], BF)        # 0:64 B^T, 64:128 C^T
            Ct2 = res.tile([128, T], BF)        # C^T duplicated on both halves
            xsTok = res.tile([128, NCH, 2048], BF)
            gb = res.tile([128, NCH, 2048], BF)          # z, then gated y
            Btok = res.tile([128, NCH, 64], BF)
            # decay tensors, packed layout [c*32+h, q]
            dt4 = res.tile([128, Q], F32)       # raw dt proj, then softplus
            lndt = res.tile([128, Q], F32)
            cum = res.tile([128, Q], F32)
            clg = res.tile([128, Q], F32)       # cum - ln dt
            negclT = res.tile([128, 128], F16)  # [q, c*32+h] = ln dt - cum
            dstateT = res.tile([128, 128], BF)   # [q, c*32+h]
            lam = res.tile([128, NJ, NCH], BF)   # pair-packed chunk decay
            Pp = res.tile([128, NCH, NJ], F32)   # cumulative lam products
            Snp = res.tile([128, NCH * NJ, 64], BF)
            yi = res.tile([128, NCH, 2048], BF, tag="bigT")  # intra-chunk y
            ss_sb = res.tile([128, NCH], F32)
            rs_sb = res.tile([128, NCH], F32)
            ones4 = res.tile([128, Q], F32)
            nc.vector.memset(ones4[:], 1.0)

            # ---- dt projection + decay chain (emitted FIRST) ---------------
            pdt = ps_main.tile([HN, T], F32, tag="mm", name="pdt")
            for kt in range(NKT):
                nc.tensor.matmul(pdt[:], wdt[:, kt, :], xT[:, kt, :],
                                 start=(kt == 0), stop=(kt == NKT - 1))
            dtsb = work.tile([HN, T], F32, tag="dtsb", bufs=1, name="dtsb")
            nc.scalar.copy(dtsb[:], pdt[:])
            for c in range(NCH):
                nc.sync.dma_start(dt4[c * 32:(c + 1) * 32, :],
                                  dtsb[:, c * Q:(c + 1) * Q])
            # softplus: dt = ln(1 + exp(raw + dtb))
            nc.scalar.activation(dt4[:], dt4[:], AF.Exp, bias=dtb4[:])
            nc.scalar.activation(dt4[:], dt4[:], AF.Ln, bias=1.0)
            nc.scalar.activation(lndt[:], dt4[:], AF.Ln)
            nc.vector.tensor_scalar_mul(dt4[:], dt4[:], negA4[:])
            nc.vector.tensor_tensor_scan(
                cum[:], ones4[:], dt4[:], 0.0, OP.mult, OP.add)
            nc.vector.tensor_tensor(clg[:], cum[:], lndt[:], OP.subtract)
            ptr_cl = ps_tr.tile([128, 128], F32, tag="trd", bufs=1, name="ptr_cl")
            nc.tensor.transpose(ptr_cl[:], clg[:], idf[:])
            nc.vector.tensor_scalar_mul(negclT[:], ptr_cl[:], -1.0)
            dscm = work.tile([128, Q], F32, tag="dscm", bufs=1, name="dscm")
            nc.scalar.activation(dscm[:], clg[:], AF.Exp,
                                 bias=cum[:, Q - 1:Q], scale=-1.0)
            ptr_ds = ps_tr.tile([128, 128], F32, tag="trd", bufs=1, name="ptr_ds")
            nc.tensor.transpose(ptr_ds[:], dscm[:], idf[:])
            nc.vector.tensor_copy(dstateT[:], ptr_ds[:])
            ecum_sb = work.tile([128, Q], BF, tag="ecum", bufs=1, name="ecum_sb")
            nc.scalar.activation(ecum_sb[:], cum[:], AF.Exp)
            nc.sync.dma_start(ecum_dram[:, :], ecum_sb[:])
            cum16 = work.tile([128, Q], F16, tag="cum16", bufs=1, name="cum16")
            nc.vector.tensor_copy(cum16[:], cum[:])
            nc.sync.dma_start(cumh_dram[:, :], cum16[:])

            # lam[p, j, c] = ecum[c*32 + j (+16 hi-half), Q-1] via stride-0 DMA
            for c in range(NCH):
                nc.sync.dma_start(
                    lam[0:64, :, c],
                    bass.AP(tensor=ecum_dram, offset=c * 32 * Q + Q - 1,
                            ap=[[0, 64], [Q, NJ]]))
                nc.sync.dma_start(
                    lam[64:128, :, c],
                    bass.AP(tensor=ecum_dram, offset=(c * 32 + 16) * Q + Q - 1,
                            ap=[[0, 64], [Q, NJ]]))
            nc.vector.tensor_copy(Pp[:, 0, :], lam[:, :, 0])
            for c in range(1, NCH):
                nc.vector.tensor_tensor(Pp[:, c, :], Pp[:, c - 1, :],
                                        lam[:, :, c], OP.mult)

            # ---- in_proj: xBC block (channel-major) ------------------------
            for ct in range(CONVT):
                pt = ps_main.tile([128, T], F32, tag="mm", name="pxbc")
                for kt in range(NKT):
                    nc.tensor.matmul(
                        pt[:], wxbc[:, kt, ct * 128:(ct + 1) * 128],
                        xT[:, kt, :], start=(kt == 0), stop=(kt == NKT - 1))
                nc.vector.tensor_copy(xbcT[:, ct, 3:3 + T], pt[:])

            # ---- conv via PE diagonal matmuls + silu -----------------------
            # diagonal conv-weight tiles built on device from the identity
            conv_out = []
            for ct in range(CONVT):
                dw = wx.tile([128, 4, 128], BF, tag="dw", bufs=2, name="dw")
                for k in range(4):
                    nc.vector.tensor_scalar_mul(dw[:, k, :], idb[:],
                                                cwT[:, k, ct:ct + 1])
                cp = ps_main.tile([128, T], F32, tag="mm", name="pconv")
                for k in range(4):
                    nc.tensor.matmul(cp[:], dw[:, k, :],
                                     xbcT[:, ct, k:k + T],
                                     start=(k == 0), stop=(k == 3))
                dest = BCt
                if ct < 16:
                    xs_scr = wx.tile([128, T], BF, tag="xs_scr")
                    conv_out.append(xs_scr)
                    dest = xs_scr
                nc.scalar.activation(dest[:], cp[:], AF.Silu,
                                     bias=convb[:, ct:ct + 1])
            nc.sync.dma_start(Ct2[0:64, :], BCt[64:128, :])
            nc.sync.dma_start(Ct2[64:128, :], BCt[64:128, :])

            # ---- xs transposes -> token-major xsTok ------------------------
            for ct in range(16):
                xs_scr = conv_out[ct]
                ptr = ps_tr.tile([128, 512], BF, tag="trb")
                for i in range(NCH):
                    nc.tensor.transpose(
                        ptr[:, i * 128:(i + 1) * 128],
                        xs_scr[:, i * 128:(i + 1) * 128], idb[:])
                nc.scalar.copy(
                    xsTok[:, :, ct * 128:(ct + 1) * 128],
                    ptr[:].rearrange("p (i q) -> p i q", i=NCH))
            # B transposes: [64, T] -> Btok [128, NCH, 64]
            ptrB = ps_tr.tile([128, 512], BF, tag="trb")
            for c in range(NCH):
                nc.tensor.transpose(
                    ptrB[:, c * 128:c * 128 + 64],
                    BCt[0:64, c * 128:(c + 1) * 128], idb[0:64, 0:64])
            nc.scalar.copy(
                Btok[:, :, :],
                ptrB[:].rearrange("p (i q) -> p i q", i=NCH)[:, :, 0:64])

            # ---- Loop1: local chunk states (zero entering state) -----------
            for c in range(NCH):
                Btil = work.tile([128, HN, 64], BF, tag="Btil", bufs=1, name="Btil")
                nc.vector.tensor_tensor(
                    Btil[:],
                    bass.AP(tensor=Btok.tensor, offset=Btok[:, c, :].offset,
                            ap=[Btok.ap[0], [0, HN], [1, 64]]),
                    bass.AP(tensor=dstateT.tensor,
                            offset=dstateT[:, c * 32:(c + 1) * 32].offset,
                            ap=[dstateT.ap[0], [1, HN], [0, 64]]),
                    OP.mult)
                for half in range(2):
                    pu = ps_y.tile([128, 8, 64], F32, tag="py", name="pu")
                    for j in range(8):
                        jj = half * 8 + j
                        for par in range(2):
                            h = 16 * par + jj
                            nc.tensor.matmul(
                                pu[par * 64:par * 64 + 64, j, :],
                                Btil[:, h, :],
                                xsTok[:, c, h * 64:(h + 1) * 64],
                                start=True, stop=True)
                    jsl = slice(half * 8, (half + 1) * 8)
                    if c == 0:
                        nc.vector.tensor_copy(Snp[:, jsl, :], pu[:])
                    else:
                        tmp = work.tile([128, 8, 64], BF, tag="stmp")
                        nc.vector.tensor_tensor(
                            tmp[:], Snp[:, (c - 1) * NJ + half * 8:(c - 1) * NJ + half * 8 + 8, :],
                            bass.AP(tensor=lam.tensor,
                                    offset=lam[:, jsl, c].offset,
                                    ap=[lam.ap[0], [NCH, 8], [0, 64]]),
                            OP.mult)
                        nc.vector.tensor_tensor(
                            Snp[:, c * NJ + half * 8:c * NJ + half * 8 + 8, :],
                            tmp[:], pu[:], OP.add)

            # ---- boundary state AllReduce over token-half pairs ------------
            with tc.high_priority():
                ar_in = work.tile([128, NJ * 64], BF, tag="ario", bufs=1,
                                  name="ar_in")
                nc.vector.tensor_scalar_mul(
                    ar_in[:], Snp[:, (NCH - 1) * NJ:NCH * NJ, :].rearrange("p j q -> p (j q)"),
                    smask[:])
                bb_in = dram.tile([128, NJ * 64], BF)
                bb_out = dram.tile([128, NJ * 64], BF)
                nc.gpsimd.dma_start(bb_in[:], ar_in[:])
                nc.gpsimd.collective_compute(
                    "AllReduce", OP.add, replica_groups=PAIR_GROUPS,
                    ins=[bb_in.opt()], outs=[bb_out.opt()])

            # ---- z proj + Loop2 (W construction + intra-chunk y),
            #      interleaved per chunk; hides the collective ---------------
            for c in range(NCH):
                for nt in range(4):
                    pz = ps_main.tile([128, T], F32, tag="mm", name="pz")
                    for kt in range(NKT):
                        nc.tensor.matmul(
                            pz[:], xT[:, kt, c * 128:(c + 1) * 128],
                            wz[:, kt, nt * 512:(nt + 1) * 512],
                            start=(kt == 0), stop=(kt == NKT - 1))
                    nc.vector.tensor_copy(gb[:, c, nt * 512:(nt + 1) * 512],
                                          pz[:])
                sl = slice(c * Q, (c + 1) * Q)
                pg = ps_y.tile([128, 128], F32, tag="py", name="pg")
                nc.tensor.matmul(pg[:], BCt[0:64, sl], Ct2[0:64, sl],
                                 start=True, stop=True)
                gsb = work.tile([128, 128], BF, tag="gsb", name="gsb")
                nc.vector.tensor_tensor(gsb[:], pg[:], tri01[:], OP.mult)
                Dxs = work.tile([128, 2048], BF, tag="dxs", bufs=1, name="Dxs")
                nc.vector.tensor_tensor(Dxs[:], xsTok[:, c, :], Dexp[:],
                                        OP.mult)
                crow = work.tile([128, HN, Q], F16, tag="crow", bufs=1,
                                 name="crow")
                nc.sync.dma_start(
                    crow[:],
                    bass.AP(tensor=cumh_dram, offset=c * 32 * Q,
                            ap=[[0, 128], [Q, HN], [1, Q]]))
                nc.vector.tensor_tensor(
                    crow[:], crow[:],
                    bass.AP(tensor=negclT.tensor,
                            offset=negclT[:, c * 32:(c + 1) * 32].offset,
                            ap=[negclT.ap[0], [1, HN], [0, Q]]),
                    OP.add)
                WT = work.tile([128, HN, Q], BF, tag="wt", bufs=1, name="WT")
                nc.vector.tensor_scalar_min(WT[:], crow[:], 30.0)
                nc.scalar.activation(WT[:], WT[:], AF.Exp)
                nc.vector.tensor_tensor(
                    WT[:],
                    bass.AP(tensor=gsb.tensor, offset=gsb.offset,
                            ap=[gsb.ap[0], [0, HN], gsb.ap[1]]),
                    WT[:], OP.mult)
                for hh in range(4):
                    py = ps_y.tile([128, 8, 64], F32, tag="py", name="py")
                    for i in range(8):
                        h = hh * 8 + i
                        nc.tensor.matmul(py[:, i, :], WT[:, h, :],
                                         xsTok[:, c, h * 64:(h + 1) * 64],
                                         start=True, stop=True)
                    nc.vector.tensor_tensor(
                        yi[:, c, hh * 512:(hh + 1) * 512],
                        py[:].rearrange("p j q -> p (j q)"),
                        Dxs[:, hh * 512:(hh + 1) * 512], OP.add)

            # late weights alias the in_proj/z weights' space
            wout = res.tile([128, 16, 1024], BF, tag="w1")
            wout_r = wout_d.rearrange("k p t -> p k t")
            for i in range(4):
                nc.sync.dma_start(wout[:, i * 4:(i + 1) * 4, :],
                                  wout_r[:, i * 4:(i + 1) * 4, :])
            wo = res.tile([128, 8, 1024], BF, tag="w2")
            wo_r = wo_d.rearrange("k p t -> p k t")
            for i in range(2):
                nc.sync.dma_start(wo[:, i * 4:(i + 1) * 4, :],
                                  wo_r[:, i * 4:(i + 1) * 4, :])

            # ---- collective result: entering state per chunk ---------------
            s_in = work.tile([128, NJ * 64], BF, tag="sins", bufs=1,
                             name="s_in")
            nc.sync.dma_start(s_in[:], bb_out[:])
            seff = work.tile([128, NJ, 64], BF, tag="seff", bufs=1,
                             name="seff")
            nc.vector.tensor_scalar_mul(
                seff[:], s_in[:].rearrange("p (j q) -> p j q", j=NJ),
                rmask[:])
            # Snp[c] <- Snp[c-1] + Pp[c-1] * seff ; Snp[0] <- seff
            for c in range(NCH - 1, 0, -1):
                tmp = work.tile([128, NJ, 64], BF, tag="sutmp", bufs=1)
                nc.vector.tensor_tensor(
                    tmp[:], seff[:],
                    bass.AP(tensor=Pp.tensor, offset=Pp[:, c - 1, :].offset,
                            ap=[Pp.ap[0], [1, NJ], [0, 64]]),
                    OP.mult)
                nc.vector.tensor_tensor(Snp[:, c * NJ:(c + 1) * NJ, :],
                                        tmp[:],
                                        Snp[:, (c - 1) * NJ:c * NJ, :],
                                        OP.add)
            nc.vector.tensor_copy(Snp[:, 0:NJ, :], seff[:])

            # ---- Loop3+Loop4 per chunk: corrections, gating, out_proj, Wo --
            for c in range(NCH):
                sl = slice(c * Q, (c + 1) * Q)
                # Ctilde pair-packed: C^T * exp(cum) per head
                ecrow = work.tile([128, NJ, Q], BF, tag="ecrow", bufs=1, name="ecrow")
                nc.sync.dma_start(
                    ecrow[0:64, :, :],
                    bass.AP(tensor=ecum_dram, offset=c * 32 * Q,
                            ap=[[0, 64], [Q, NJ], [1, Q]]))
                nc.sync.dma_start(
                    ecrow[64:128, :, :],
                    bass.AP(tensor=ecum_dram, offset=(c * 32 + 16) * Q,
                            ap=[[0, 64], [Q, NJ], [1, Q]]))
                Ctil = ecrow
                nc.vector.tensor_tensor(
                    Ctil[:],
                    bass.AP(tensor=Ct2.tensor, offset=Ct2[:, sl].offset,
                            ap=[Ct2.ap[0], [0, NJ], [1, Q]]),
                    ecrow[:], OP.mult)
                sgz = work.tile([128, 2048], BF, tag="sgz", bufs=1, name="sgz")
                nc.scalar.activation(sgz[:], gb[:, c, :], AF.Silu)
                for hh in range(4):
                    py2 = ps_y.tile([128, 8, 64], F32, tag="py", name="py2")
                    for i in range(8):
                        h = hh * 8 + i
                        par = h // 16
                        nc.tensor.matmul(
                            py2[:, i, :],
                            Ctil[par * 64:par * 64 + 64, h % 16, :],
                            Snp[par * 64:par * 64 + 64, c * NJ + h % 16, :],
                            start=True, stop=True)
                    hsl = slice(hh * 512, (hh + 1) * 512)
                    ysum = work.tile([128, 512], BF, tag="ysum", name="ysum")
                    nc.vector.tensor_tensor(
                        ysum[:], py2[:].rearrange("p j q -> p (j q)"),
                        yi[:, c, hsl], OP.add)
                    nc.vector.tensor_tensor(gb[:, c, hsl], ysum[:],
                                            sgz[:, hsl], OP.mult)
                sq = work.tile([128, 2048], BF, tag="dxs", bufs=1, name="sq")
                nc.vector.scalar_tensor_tensor(
                    sq[:], gb[:, c, :], 1.0, gb[:, c, :], OP.mult, OP.mult,
                    accum_out=ss_sb[:, c:c + 1])
                # per-chunk rs = 1/sqrt(mean(y^2) + eps)
                nc.vector.tensor_scalar(rs_sb[:, c:c + 1], ss_sb[:, c:c + 1],
                                        1.0 / 2048.0, 1e-5, OP.mult, OP.add)
                nc.scalar.activation(rs_sb[:, c:c + 1], rs_sb[:, c:c + 1],
                                     AF.Sqrt)
                nc.vector.reciprocal(rs_sb[:, c:c + 1], rs_sb[:, c:c + 1])

                # out_proj: transpose gb chunk, matmul with wout, scale by rs
                gbt = work.tile([128, 16, 128], BF, tag="gbt", bufs=1, name="gbt")
                for pg_i in range(4):
                    ptr = ps_tr.tile([128, 512], BF, tag="trb")
                    for i in range(4):
                        hp = pg_i * 4 + i
                        nc.tensor.transpose(
                            ptr[:, i * 128:(i + 1) * 128],
                            gb[:, c, hp * 128:(hp + 1) * 128], idb[:])
                    nc.vector.tensor_copy(
                        gbt[:, pg_i * 4:(pg_i + 1) * 4, :],
                        ptr[:].rearrange("p (i q) -> p i q", i=4))
                Pn = work.tile([128, 1024], BF, tag="Pn", bufs=1, name="Pn")
                for nt in range(2):
                    po = ps_main.tile([128, T], F32, tag="mm", name="po")
                    for kt in range(16):
                        nc.tensor.matmul(
                            po[:], gbt[:, kt, :],
                            wout[:, kt, nt * 512:(nt + 1) * 512],
                            start=(kt == 0), stop=(kt == 15))
                    nc.vector.tensor_scalar_mul(
                        Pn[:, nt * 512:(nt + 1) * 512], po[:],
                        rs_sb[:, c:c + 1])
                # Wo block: transpose Pn, matmul with wo
                pnT = work.tile([128, 8, 128], BF, tag="gbt", bufs=1, name="pnT")
                for pg_i in range(2):
                    ptr = ps_tr.tile([128, 512], BF, tag="trb")
                    for i in range(4):
                        hp = pg_i * 4 + i
                        nc.tensor.transpose(
                            ptr[:, i * 128:(i + 1) * 128],
                            Pn[:, hp * 128:(hp + 1) * 128], idb[:])
                    nc.scalar.copy(
                        pnT[:, pg_i * 4:(pg_i + 1) * 4, :],
                        ptr[:].rearrange("p (i q) -> p i q", i=4))
                for nt in range(2):
                    co = ps_main.tile([128, T], F32, tag="mm", name="co")
                    for kt in range(8):
                        nc.tensor.matmul(
                            co[:], pnT[:, kt, :],
                            wo[:, kt, nt * 512:(nt + 1) * 512],
                            start=(kt == 0), stop=(kt == 7))
                    cof = work.tile([128, 512], F32, tag="cof", bufs=1, name="cof")
                    nc.vector.tensor_copy(cof[:], co[:])
                    nc.sync.dma_start(C_d[c, :, nt * 512:(nt + 1) * 512],
                                      cof[:])

    nc.compile()
    return nc


# ----------------------------------------------------------------------------
# Host orchestration
# ----------------------------------------------------------------------------
_cache = {}


def _get_core_nc():
    if "core" not in _cache:
        _cache["core"] = build_core()
    return _cache["core"]


def _prep_core_inputs(x_seq, Win, convw, convb, dtb, Alog, Dsk, normw, Wout,
                      Wo, d_idx, h):
    """Host-side shard/cast for one (dir, batch, token-half) core."""
    z_cols = np.arange(0, 2048)
    xs_cols = np.arange(2048, 4096)
    bc_cols = np.arange(4096, 4224)
    dt_cols = np.arange(4224, 4256)
    xsbc_cols = np.concatenate([xs_cols, bc_cols])

    toks = x_seq[h * T:(h + 1) * T]                             # [512, 1024]
    if h == 0:
        xbch = np.zeros((3, CONVT * 128), np.float32)
    else:
        xbch = x_seq[h * T - 3:h * T] @ Win[:, xsbc_cols]       # [3, 2176]

    wxbc = Win[:, xsbc_cols]                                    # [1024, 2176]
    wz = Win[:, z_cols]
    wdt = Win[:, dt_cols]
    wout = normw[:, None] * Wout                                # [2048, 1024]
    wo_blk = Wo[d_idx * 1024:(d_idx + 1) * 1024]                # [1024, 1024]

    cw = convw[:, 0, :]                                         # [4, 2176]
    cb = convb                                                  # [2176]
    cwT = np.ascontiguousarray(
        cw.reshape(4, CONVT, 128).transpose(2, 0, 1))           # [128, 4, 17]
    tri01 = (np.arange(128)[None, :] >= np.arange(128)[:, None]).astype(BF16)
    mval = 1.0 if h == 0 else 0.0

    return {
        "xT": np.ascontiguousarray(toks.T.reshape(NKT, 128, T).astype(BF16)),
        "xbch": np.ascontiguousarray(
            xbch.T.reshape(CONVT, 128, 3).astype(BF16)),
        "wxbc": np.ascontiguousarray(
            wxbc.reshape(NKT, 128, CONVT * 128).astype(BF16)),
        "wz": np.ascontiguousarray(wz.reshape(NKT, 128, 2048).astype(BF16)),
        "wdt": np.ascontiguousarray(wdt.reshape(NKT, 128, HN).astype(BF16)),
        "wout": np.ascontiguousarray(wout.reshape(16, 128, 1024).astype(BF16)),
        "wo": np.ascontiguousarray(wo_blk.reshape(8, 128, 1024).astype(BF16)),
        "cwT": cwT.astype(np.float32),
        "convb": np.ascontiguousarray(cb.reshape(CONVT, 128)).astype(np.float32),
        "dtb": np.tile(np.asarray(dtb, np.float32), NCH),
        "negA": np.tile((-np.exp(Alog)).astype(np.float32), NCH),
        "Dexp": np.repeat(np.asarray(Dsk, np.float32), 64).astype(BF16),
        "tri01": tri01,
        "idf": np.eye(128, dtype=np.float32),
        "idb": np.eye(128, dtype=np.float32).astype(BF16),
        "smask": np.full((128, 1), mval, np.float32),
        "rmask": np.full((128, 1), 1.0 - mval, np.float32),
    }


def _build_in_maps(inputs):
    x = np.asarray(inputs["x"], dtype=np.float32)               # [2, L, DM]
    Wo = np.asarray(inputs["Wo"], dtype=np.float32)
    dir_params = {}
    for pre in ("f", "b"):
        dir_params[pre] = tuple(np.asarray(inputs[pre + k], dtype=np.float32)
                                for k in ("Win", "convw", "convb", "dtb",
                                          "Alog", "D", "normw", "Wout"))
    # core order: id = b*4 + d*2 + h
    core_keys = [(b, d, h) for b in range(2) for d in range(2)
                 for h in range(2)]
    in_maps = []
    for (b, d, h) in core_keys:
        pre = "f" if d == 0 else "b"
        x_seq = x[b] if d == 0 else x[b, ::-1]
        in_maps.append(_prep_core_inputs(x_seq, *dir_params[pre], Wo, d, h))
    return in_maps, core_keys


def kernel(**inputs):
    in_maps, core_keys = _build_in_maps(inputs)
    nc = _get_core_nc()
    res = bass_utils.run_bass_kernel_spmd(nc, in_maps, core_ids=list(range(8)))

    bo = np.asarray(inputs["bo"], dtype=np.float32)
    out = np.zeros((2, L, DM), np.float32)
    for ci, (b, d, h) in enumerate(core_keys):
        Cp = res.results[ci]["C"].reshape(T, 1024)
        if d == 0:
            out[b, h * T:(h + 1) * T] += Cp
        else:
            # bwd core's tokens are reversed-sequence [h*T:(h+1)*T]
            out[b, L - (h + 1) * T:L - h * T] += Cp[::-1]
    out += bo[None, None, :]
    return out.astype(np.float32)
